# revision 14
# baseline (speedup 1.0000x reference)
"""Trainium2 Bass kernel for nn_CrossAttentionProjectLayer.

Reference computation (per batch b):
    k = enc @ Wk.T + bk                       [S, E] -> [S, H, D]
    v = enc @ Wv.T + bv                       [S, E] -> [S, H, D]
    proj[s,h,kk] = sum_d k[s,h,d] * (D^-.5/tau^2) * random_matrices[h,kk,d]
    phi = [sin(proj), cos(proj)] * K^-.5, zeroed where mask
    s[h,kk,d] = sum_s phi[s,h,kk] * v[s,h,d]
    z[h,kk]   = sum_s phi[s,h,kk]
    rm = random_matrices / tau

Device strategy (8 cores, batch-parallel, one batch per core):
  - Host folds Wk/rm/scales into one matrix C[e, h*kk] and bias c0, so the
    device computes proj = enc @ C + c0 directly.  C is concatenated with
    Wv.T into CW[e, 2048]; one fused GEMM produces proj and v.
  - enc is passed pre-transposed [E, S] so contraction (e) lands on SBUF
    partitions with no on-device transposes.
  - phi (sin|cos, bf16) is the stationary operand of a per-head matmul
    against [v | 1] (65 cols, bf16) accumulating [s_h | z_h] in PSUM over
    all token tiles.  mask and the K^-.5 scale are folded into a per-token
    scalar applied to the [v | 1] tile.
"""

import os
from contextlib import ExitStack

import numpy as np

import concourse.bass as bass
import concourse.mybir as mybir
import concourse.tile as tile
from concourse import bacc
from concourse.bass_utils import run_bass_kernel_spmd

S, B, E = 4096, 8, 1024
H, D, K = 16, 64, 64
TAU = 1.0
NCORES = 8
PI = float(np.pi)

f32 = mybir.dt.float32
f32r = mybir.dt.float32r
bf16 = mybir.dt.bfloat16

# Heads packed 7/7/2 into three PSUM banks ([128, 65] region per head).
HEADS_PER_BANK = 7


def _s_slot(s_ps, h):
    bank, hl = divmod(h, HEADS_PER_BANK)
    return s_ps[bank][:, hl * 65:(hl + 1) * 65]


def _bcast(ap_, p=128):
    """View a DRAM AP broadcast along a new partition dim of size p."""
    return bass.AP(tensor=ap_.tensor, offset=ap_.offset,
                   ap=[[0, p]] + [list(x) for x in ap_.ap])


def build(n_tokens=S, mm_dt="fp32r"):
    """Build the single-core SPMD program for n_tokens tokens."""
    assert n_tokens % 512 == 0
    ntiles = n_tokens // 128
    nsup = n_tokens // 512

    # fp32r must be declared end-to-end (DMA producer included) or the BIR
    # verifier rejects the matmul consumer; dt.np(float32r) is float32 so
    # the host still passes plain f32 arrays.
    io_dt = {"bf16": bf16, "fp32r": f32r}.get(mm_dt, f32)

    def mmcast(ap_):
        return ap_

    nc = bacc.Bacc("TRN2", target_bir_lowering=False, debug=False)

    encT = nc.dram_tensor("encT", [8, 128, n_tokens], io_dt, kind="ExternalInput")
    cw = nc.dram_tensor("cw", [8, 128, 2048], io_dt, kind="ExternalInput")
    c0b = nc.dram_tensor("c0b", [1024], f32, kind="ExternalInput")
    bvb = nc.dram_tensor("bvb", [1024], f32, kind="ExternalInput")
    keepc = nc.dram_tensor("keepc", [128, ntiles], f32, kind="ExternalInput")
    s_out = nc.dram_tensor("s_out", [H, 2 * K, D], f32, kind="ExternalOutput")
    z_out = nc.dram_tensor("z_out", [H, 2 * K], f32, kind="ExternalOutput")

    with tile.TileContext(nc) as tc, ExitStack() as ctx:
        const = ctx.enter_context(tc.tile_pool(name="const", bufs=1))
        encp = ctx.enter_context(tc.tile_pool(name="encp", bufs=2))
        phip = ctx.enter_context(tc.tile_pool(name="phip", bufs=2))
        pmp = ctx.enter_context(tc.tile_pool(name="pmp", bufs=2))
        vzp = ctx.enter_context(tc.tile_pool(name="vzp", bufs=2))
        outp = ctx.enter_context(tc.tile_pool(name="outp", bufs=1))
        psum = ctx.enter_context(
            tc.tile_pool(name="psum", bufs=1, space=bass.MemorySpace.PSUM))

        cw_sb = const.tile([128, 8, 2048], io_dt)
        for c in range(8):
            nc.sync.dma_start(out=cw_sb[:, c, :], in_=cw[c])
        c0_sb = const.tile([128, 16, 64], f32)
        nc.sync.dma_start(out=c0_sb, in_=_bcast(c0b.ap().rearrange(
            "(h k) -> h k", h=H)))
        bv_sb = const.tile([128, 16, 64], f32)
        nc.sync.dma_start(out=bv_sb, in_=_bcast(bvb.ap().rearrange(
            "(h d) -> h d", h=H)))
        keep_sb = const.tile([128, ntiles], f32)
        nc.sync.dma_start(out=keep_sb, in_=keepc.ap())
        hpi_sb = const.tile([128, 1], f32)
        nc.vector.memset(hpi_sb, PI / 2)
        quart_sb = const.tile([128, 1], f32)
        nc.vector.memset(quart_sb, 0.25)

        # proj in banks 0-1, v in banks 2-3
        psum_pv = psum.tile([128, 2048], f32)
        s_ps = [psum.tile([128, 455], f32, name="s_ps0"),
                psum.tile([128, 455], f32, name="s_ps1"),
                psum.tile([128, 130], f32, name="s_ps2")]
        for sp in s_ps:
            nc.vector.memset(sp, 0.0)

        def emit_s(prev):
            t, phi, vz = prev
            for h in range(H):
                nc.tensor.matmul(
                    _s_slot(s_ps, h),
                    lhsT=phi[:, h],          # [128 tok, 2, 64] -> 128 cols
                    rhs=vz[:, h],            # [128 tok, 65]
                    start=False, stop=False, skip_group_check=True)

        prev = None
        for tt in range(nsup):
            enc_sb = encp.tile([128, 8, 512], io_dt)
            for c in range(8):
                nc.sync.dma_start(out=enc_sb[:, c, :],
                                  in_=encT[c, :, tt * 512:(tt + 1) * 512])
            for m in range(4):
                t = tt * 4 + m
                # fused proj|v GEMM for this 128-token tile
                for n in range(4):
                    for c in range(8):
                        nc.tensor.matmul(
                            psum_pv[:, n * 512:(n + 1) * 512],
                            lhsT=mmcast(enc_sb[:, c, m * 128:(m + 1) * 128]),
                            rhs=mmcast(cw_sb[:, c, n * 512:(n + 1) * 512]),
                            start=(c == 0), stop=(c == 7))
                # previous tile's s/z accumulation fills the PE gap while
                # ACT/DVE produce this tile's phi/vz
                if prev is not None:
                    emit_s(prev)

                # Sin LUT domain is [-pi, pi], so range-reduce.  The GEMM
                # weights carry a 1/2pi factor, so psum holds proj/2pi.
                # u = psum + c0'; k = round_i32(u); w = u - k in [-.5, .5];
                # sin(proj) = Sin(2pi*w); cos = Sin(2pi*w2 + pi/2) with
                # k2 = round_i32(u + 1/4).
                proj_hk = psum_pv[:, 0:1024].rearrange("p (h k) -> p h k", h=H)
                u = pmp.tile([128, H, K], f32, name="u")
                nc.vector.tensor_add(u, proj_hk, c0_sb)
                k1 = pmp.tile([128, H, K], mybir.dt.int32, name="k1")
                nc.scalar.activation(out=k1, in_=u,
                                     func=mybir.ActivationFunctionType.Identity)
                k2 = pmp.tile([128, H, K], mybir.dt.int32, name="k2")
                nc.scalar.activation(out=k2, in_=u,
                                     func=mybir.ActivationFunctionType.Identity,
                                     bias=quart_sb)
                w1 = pmp.tile([128, H, K], f32, name="w1")
                nc.vector.tensor_sub(w1, u, k1)
                w2 = pmp.tile([128, H, K], f32, name="w2")
                nc.vector.tensor_sub(w2, u, k2)
                phi = phip.tile([128, H, 2, K], bf16)
                nc.scalar.activation(out=phi[:, :, 0, :], in_=w1,
                                     func=mybir.ActivationFunctionType.Sin,
                                     scale=2 * PI)
                nc.scalar.activation(out=phi[:, :, 1, :], in_=w2,
                                     func=mybir.ActivationFunctionType.Sin,
                                     scale=2 * PI, bias=hpi_sb)

                vz = vzp.tile([128, H, 65], bf16)
                v_hd = psum_pv[:, 1024:2048].rearrange("p (h d) -> p h d", h=H)
                nc.vector.tensor_add(vz[:, :, 0:64], v_hd, bv_sb)
                nc.vector.memset(vz[:, :, 64:65], 1.0)
                nc.vector.tensor_scalar_mul(vz[:, :, :], vz[:, :, :],
                                            keep_sb[:, t:t + 1])
                prev = (t, phi, vz)
        emit_s(prev)

        out_sb = outp.tile([128, H, 65], f32)
        nc.vector.tensor_copy(out_sb[:, 0:7, :],
                              s_ps[0].rearrange("p (h x) -> p h x", h=7))
        nc.vector.tensor_copy(out_sb[:, 7:14, :],
                              s_ps[1].rearrange("p (h x) -> p h x", h=7))
        nc.vector.tensor_copy(out_sb[:, 14:16, :],
                              s_ps[2].rearrange("p (h x) -> p h x", h=2))
        nc.sync.dma_start(out=s_out.ap().rearrange("h k d -> k h d"),
                          in_=out_sb[:, :, 0:64])
        nc.sync.dma_start(out=z_out.ap().rearrange("h k -> k h"),
                          in_=out_sb[:, :, 64])

    nc.compile()
    return nc


def make_in_maps(inputs, n_tokens=S, mm_dt="fp32r", cores=NCORES):
    """Host-side shard + algebraic folding. Returns per-core input dicts."""
    enc = np.asarray(inputs["encoder_output"], np.float32)
    Wk = np.asarray(inputs["Wk"], np.float64)
    bk = np.asarray(inputs["bk"], np.float64)
    Wv = np.asarray(inputs["Wv"], np.float32)
    bv = np.asarray(inputs["bv"], np.float32)
    rm = np.asarray(inputs["random_matrices"], np.float64)
    mask = np.asarray(inputs["mask"], bool)

    ntiles = n_tokens // 128
    io_np = np.float32 if mm_dt != "bf16" else None  # bf16 handled below

    # proj[s,h,kk] = sum_d k[s,h,d] * M_h[d,kk],  M_h = rm[h].T * (D^-.5/tau^2)
    # The extra 1/2pi folds the Sin range-reduction scale into the GEMM.
    scale = (D ** -0.5) / (TAU * TAU) / (2 * np.pi)
    M = np.transpose(rm, (0, 2, 1)) * scale          # [H, D, K]
    C = np.zeros((E, H * K), np.float64)
    c0 = np.zeros(H * K, np.float64)
    for h in range(H):
        WkhT = Wk[h * D:(h + 1) * D, :].T            # [E, D]
        C[:, h * K:(h + 1) * K] = WkhT @ M[h]
        c0[h * K:(h + 1) * K] = bk[h * D:(h + 1) * D] @ M[h]
    CW = np.concatenate([C.astype(np.float32), Wv.T], axis=1)  # [E, 2048]
    cw = np.ascontiguousarray(CW.reshape(8, 128, 2048))

    if mm_dt == "bf16":
        import ml_dtypes
        cw = cw.astype(ml_dtypes.bfloat16)

    in_maps = []
    for b in range(cores):
        encT = np.ascontiguousarray(enc[:n_tokens, b, :].T.reshape(8, 128, n_tokens))
        if mm_dt == "bf16":
            import ml_dtypes
            encT = encT.astype(ml_dtypes.bfloat16)
        keep = (~mask[:n_tokens, b, 0, 0]).astype(np.float32) * (K ** -0.5)
        keepc = np.ascontiguousarray(keep.reshape(ntiles, 128).T)  # [128, ntiles]
        in_maps.append({
            "encT": encT,
            "cw": cw,
            "c0b": c0.astype(np.float32),
            "bvb": bv.astype(np.float32),
            "keepc": keepc,
        })
    return in_maps


_NC_CACHE = {}
LAST_RESULTS = None


def kernel(**inputs):
    global LAST_RESULTS
    mm_dt = os.environ.get("KERNEL_MM_DT", "fp32r")
    key = (S, mm_dt)
    if key not in _NC_CACHE:
        _NC_CACHE[key] = build(S, mm_dt)
    nc = _NC_CACHE[key]

    in_maps = make_in_maps(inputs, S, mm_dt)
    res = run_bass_kernel_spmd(
        nc, in_maps, core_ids=list(range(NCORES)),
        trace=bool(os.environ.get("KERNEL_TRACE")),
    )
    LAST_RESULTS = res

    s = np.stack([res.results[b]["s_out"] for b in range(NCORES)], axis=0)
    z = np.stack([res.results[b]["z_out"] for b in range(NCORES)], axis=0)
    rm = (np.asarray(inputs["random_matrices"], np.float32) / TAU)
    return s.astype(np.float32), z.astype(np.float32), rm
